# revision 12
# baseline (speedup 1.0000x reference)
"""ECT layer (segment_reduce) Trainium2 kernel.

Math (matches the jax reference):
    nh  = x @ v                          [N, T]
    ecc = sigmoid(SCALE*(lin_r - nh))    [R, N, T]
    ect = segment_sum(ecc over N by index) -> [B, R, T]
    out = ect / max(ect over (R,T) per b)

Sharding: data-parallel over point clouds (bins). Core c owns global bins
[4c, 4c+4); the host routes every point to its bin's core, so no cross-core
reduction is needed. Each core:
  per 128-point tile:
    PE   : nh100 = 100 * x_tile @ v           (fp32 matmul, K=3)
    DVE  : z = linb - nh100  (broadcast nh over the R axis)   [128, R*T]
    ACT  : ecc = sigmoid(z)                                    [128, R*T]
    DVE  : onehot[p, b] = (idx[p] == b)                        [128, 4]
    PE   : ect += onehot.T @ ecc   (fp32r, accumulated in PSUM) [4, R*T]
  epilogue: per-bin max over R*T, divide, DMA out.
"""

import numpy as np

N = 100000
B = 32
R = 32
T = 32
D = 3
SCALE = 100.0

NCORES = 8
BLOC = B // NCORES        # local bins per core
CAP = 13312               # per-core point capacity (104 tiles of 128)
PTILE = 128
TILES = CAP // PTILE
F = R * T                 # 1024 output features per bin
FH = F // 2               # 512, max moving free dim per matmul

_cache = {}


def _build():
    """Build + bacc-compile the SPMD program once per process."""
    import concourse.tile as tile
    from concourse import bacc, mybir

    nc = bacc.Bacc("TRN2", target_bir_lowering=False, debug=False,
                   num_devices=NCORES)
    f32 = mybir.dt.float32
    f32r = mybir.dt.float32r

    xT_d = nc.dram_tensor("xT", [D, CAP], f32, kind="ExternalInput")
    vs_d = nc.dram_tensor("vs", [D, T], f32, kind="ExternalInput")
    linb_d = nc.dram_tensor("linb", [PTILE, F], f32, kind="ExternalInput")
    oh_d = nc.dram_tensor("ohT", [PTILE, TILES * BLOC], f32,
                          kind="ExternalInput")
    out_d = nc.dram_tensor("out", [BLOC, F], f32, kind="ExternalOutput")

    # fp32r accuracy probes (run once, independent of the main pipeline)
    pa_d = nc.dram_tensor("pa", [4, PTILE], f32, kind="ExternalInput")
    pw_d = nc.dram_tensor("pw", [4, FH], f32, kind="ExternalInput")
    pc_d = nc.dram_tensor("pc", [PTILE, BLOC], f32, kind="ExternalInput")
    pd_d = nc.dram_tensor("pd", [PTILE, FH], f32, kind="ExternalInput")
    pz_d = nc.dram_tensor("pz", [PTILE, FH], f32, kind="ExternalOutput")
    pseg_d = nc.dram_tensor("pseg", [BLOC, FH], f32, kind="ExternalOutput")

    with tile.TileContext(nc) as tc:
        with (
            tc.tile_pool(name="singles", bufs=1) as singles,
            tc.tile_pool(name="work", bufs=3) as work,
            tc.tile_pool(name="post", bufs=1) as post,
            tc.tile_pool(name="psnh", bufs=2, space="PSUM") as psnh,
            tc.tile_pool(name="psacc", bufs=1, space="PSUM") as psacc,
            tc.tile_pool(name="psprobe", bufs=1, space="PSUM") as psprobe,
        ):
            X = singles.tile([D, CAP], f32)
            VS = singles.tile([D, T], f32)
            LINB = singles.tile([PTILE, F], f32)
            OHF = singles.tile([PTILE, TILES * BLOC], f32)
            nc.sync.dma_start(out=X, in_=xT_d.ap())
            nc.sync.dma_start(out=VS, in_=vs_d.ap())
            nc.sync.dma_start(out=LINB, in_=linb_d.ap())
            nc.sync.dma_start(out=OHF, in_=oh_d.ap())
            OHR = singles.tile([PTILE, TILES * BLOC], f32r)
            nc.vector.tensor_copy(out=OHR, in_=OHF)

            ect = psacc.tile([BLOC, F], f32)

            for i in range(TILES):
                nh_ps = psnh.tile([PTILE, T], f32)
                nc.tensor.matmul(
                    out=nh_ps,
                    lhsT=X[:, i * PTILE:(i + 1) * PTILE],
                    rhs=VS,
                    start=True, stop=True,
                )
                nh = work.tile([PTILE, T], f32)
                nc.vector.tensor_copy(out=nh, in_=nh_ps)

                # z[p, r, t] = linb[p, r, t] - nh[p, t]
                z = work.tile([PTILE, F], f32)
                z3 = z.rearrange("p (r t) -> p r t", r=R)
                linb3 = LINB.rearrange("p (r t) -> p r t", r=R)
                nh3 = nh.rearrange("p (r t) -> p r t", r=1).broadcast_to(
                    [PTILE, R, T])
                nc.vector.tensor_tensor(
                    out=z3, in0=linb3, in1=nh3,
                    op=mybir.AluOpType.subtract,
                )

                ecc = work.tile([PTILE, F], f32r)
                nc.scalar.activation(
                    out=ecc, in_=z,
                    func=mybir.ActivationFunctionType.Sigmoid,
                )

                for h in range(2):
                    nc.tensor.matmul(
                        out=ect[:, h * FH:(h + 1) * FH],
                        lhsT=OHR[:, i * BLOC:(i + 1) * BLOC],
                        rhs=ecc[:, h * FH:(h + 1) * FH],
                        start=(i == 0), stop=(i == TILES - 1),
                    )

            # normalize: out = ect / max(ect, axis=free)
            mx = post.tile([BLOC, 1], f32)
            nc.vector.tensor_reduce(
                out=mx, in_=ect,
                axis=mybir.AxisListType.X, op=mybir.AluOpType.max,
            )
            rmx = post.tile([BLOC, 1], f32)
            nc.vector.reciprocal(out=rmx, in_=mx)
            outn = post.tile([BLOC, F], f32)
            nc.vector.tensor_scalar(
                out=outn, in0=ect,
                scalar1=rmx, scalar2=None,
                op0=mybir.AluOpType.mult,
            )
            nc.sync.dma_start(out=out_d.ap(), in_=outn)

            # ---- fp32r probes ----
            PA = post.tile([4, PTILE], f32)
            PW = post.tile([4, FH], f32)
            PC = post.tile([PTILE, BLOC], f32)
            PD = post.tile([PTILE, FH], f32)
            nc.sync.dma_start(out=PA, in_=pa_d.ap())
            nc.sync.dma_start(out=PW, in_=pw_d.ap())
            nc.sync.dma_start(out=PC, in_=pc_d.ap())
            nc.sync.dma_start(out=PD, in_=pd_d.ap())
            PAr = post.tile([4, PTILE], f32r)
            PWr = post.tile([4, FH], f32r)
            PCr = post.tile([PTILE, BLOC], f32r)
            PDr = post.tile([PTILE, FH], f32r)
            nc.vector.tensor_copy(out=PAr, in_=PA)
            nc.vector.tensor_copy(out=PWr, in_=PW)
            nc.vector.tensor_copy(out=PCr, in_=PC)
            nc.vector.tensor_copy(out=PDr, in_=PD)
            pz_ps = psprobe.tile([PTILE, FH], f32)
            nc.tensor.matmul(out=pz_ps, lhsT=PAr,
                             rhs=PWr, start=True, stop=True)
            pz_sb = post.tile([PTILE, FH], f32)
            nc.vector.tensor_copy(out=pz_sb, in_=pz_ps)
            nc.sync.dma_start(out=pz_d.ap(), in_=pz_sb)

            pseg_ps = psprobe.tile([BLOC, FH], f32)
            nc.tensor.matmul(out=pseg_ps, lhsT=PCr,
                             rhs=PDr, start=True, stop=True)
            pseg_sb = post.tile([BLOC, FH], f32)
            nc.vector.tensor_copy(out=pseg_sb, in_=pseg_ps)
            nc.sync.dma_start(out=pseg_d.ap(), in_=pseg_sb)

    nc.compile()
    return nc


def _host_prep(x, v, lin, index):
    """Route points to their bin's core; build per-core input maps."""
    lin100 = (SCALE * np.asarray(lin, dtype=np.float32)).reshape(R)
    linb_row = np.repeat(lin100, T)                      # [F], f = r*T + t
    linb = np.ascontiguousarray(np.broadcast_to(linb_row, (PTILE, F)),
                                dtype=np.float32)
    vs = np.ascontiguousarray(SCALE * np.asarray(v, dtype=np.float32))

    order = np.argsort(index, kind="stable")
    counts = np.bincount(index, minlength=B)
    group_counts = counts.reshape(NCORES, BLOC).sum(axis=1)
    if group_counts.max() > CAP:
        return None  # fall back to host compute
    starts = np.concatenate([[0], np.cumsum(group_counts)[:-1]])

    # probe data (same for every core)
    rng = np.random.default_rng(0)
    pa = rng.standard_normal((4, PTILE)).astype(np.float32)
    pa[3] = 1.0
    pw = np.empty((4, FH), dtype=np.float32)
    for k in range(3):
        pw[k] = np.tile(-SCALE * np.asarray(v, dtype=np.float32)[k], FH // T)
    pw[3] = linb_row[:FH]
    pc = (rng.integers(0, BLOC, PTILE)[:, None]
          == np.arange(BLOC)[None, :]).astype(np.float32)
    pd = (1.0 / (1.0 + np.exp(-rng.standard_normal((PTILE, FH))))
          ).astype(np.float32)

    in_maps = []
    for c in range(NCORES):
        pts = order[starts[c]:starts[c] + group_counts[c]]
        n_c = len(pts)
        xT = np.zeros((D, CAP), dtype=np.float32)
        xT[:, :n_c] = np.asarray(x, dtype=np.float32)[pts].T
        idxf = np.full(CAP, -1, dtype=np.int64)
        idxf[:n_c] = index[pts] - c * BLOC
        # ohT[p, i*BLOC + b] = 1.0 iff point (i*PTILE + p) is in local bin b
        oh = (idxf.reshape(TILES, PTILE)[:, :, None]
              == np.arange(BLOC)[None, None, :]).astype(np.float32)
        ohT = np.ascontiguousarray(
            oh.transpose(1, 0, 2).reshape(PTILE, TILES * BLOC))
        in_maps.append({
            "xT": xT, "vs": vs, "linb": linb, "ohT": ohT,
            "pa": pa, "pw": pw, "pc": pc, "pd": pd,
        })
    probes = {"pa": pa, "pw": pw, "pc": pc, "pd": pd}
    return in_maps, probes


def _host_fallback(x, v, lin, index):
    """Pure-numpy reference path (pathological index distributions only)."""
    x = np.asarray(x, dtype=np.float32)
    v = np.asarray(v, dtype=np.float32)
    lin = np.asarray(lin, dtype=np.float32).reshape(R, 1, 1)
    ect = np.zeros((B, R, T), dtype=np.float32)
    for s in range(0, len(x), 4096):
        xc = x[s:s + 4096]
        ic = index[s:s + 4096]
        nh = xc @ v                                   # [n, T]
        z = SCALE * (lin - nh[None, :, :])            # [R, n, T]
        ecc = 1.0 / (1.0 + np.exp(-z))
        np.add.at(ect, ic, np.transpose(ecc, (1, 0, 2)).astype(np.float32))
    return ect / ect.max(axis=(1, 2), keepdims=True)


def kernel(x, v, lin, index):
    from concourse import bass_utils

    x = np.asarray(x)
    v = np.asarray(v)
    lin = np.asarray(lin)
    index = np.asarray(index)

    prep = _host_prep(x, v, lin, index)
    if prep is None:
        return _host_fallback(x, v, lin, index)
    in_maps, _ = prep

    if "nc" not in _cache:
        _cache["nc"] = _build()
    nc = _cache["nc"]

    res = bass_utils.run_bass_kernel_spmd(nc, in_maps, list(range(NCORES)))
    out = np.concatenate(
        [res.results[c]["out"].reshape(BLOC, R, T) for c in range(NCORES)],
        axis=0,
    )
    return out.astype(np.float32)
